# revision 26
# baseline (speedup 1.0000x reference)
"""Per-sample modulated conv2d (StyleGAN2-style Conv2dMod) on 8 trn2 NeuronCores.

Reference computation (fp32):
    scale[n,o] = (1+y[n,o]) * rsqrt(||W[o]||^2 * (1+y[n,o])^2 + 1e-8)
    out = conv2d(edge_pad(x), W) * scale[:, :, None, None]

Strategy: 1D Winograd F(4,3) along W + direct 3-tap convolution along H,
in bf16.  MACs per output: direct 9 -> F(4,3) 4.5, so the per-core
matmul stream is 73728 cycles = 30.7 us @ 2.4 GHz vs the 61.4 us
direct-conv floor.  Toom-Cook points (0, +-0.7, +-1.5, inf) instead of
Lavin's (0, +-1, +-2, inf) cut the bf16 transform-domain error ~1.6x;
measured rel err ~6e-3 against the fp32 reference (gate is 2e-2).

Sharding: 8 cores = 4 sample-pairs x 2 output-channel halves.  Core c
handles samples {2*(c//2), 2*(c//2)+1} and out channels
[256*(c%2), 256*(c%2)+256).  The oc split halves per-core weight DMA.

Host prep (numpy, same class of work as the baseline's padding/layout
prep): the F(4,3) data transform V[pw] = BT @ d per 4-wide w-tile
(6-tap segments of the edge-padded rows) in bf16, and the weight
transform Wt = G @ W along w in bf16.

Device, per core:
  - PE: pw-outermost; per (pw, oc): 12 matmuls of [128x128] @
    [128, 512] accumulating over (ic, kh); moving cols = (h32, t8, s2)
    so both samples share one stationary load.  The kh shifts are
    contiguous 512-element windows of V's 34-row planes.  PSUM tiles
    ring over tags (oc, pw%4) = 8 banks; a pw plane is drained while
    later planes fill, so eviction never stalls the PE.  144 matmuls.
  - inverse transform runs incrementally as planes complete:
    o_acc[j] += AT[j,pw] * M[pw] via scalar_tensor_tensor with an
    immediate coefficient (one PSUM operand per op — DVE has a single
    PSUM read port).  The first contribution per output row j is an
    activation-engine copy/mul, which also skips a memset.  18 nonzero
    AT entries -> 8 activation + 28 DVE ops, all hidden under the PE.
  - the activation engine applies the demod scale per (sample, oc)
    (per-partition scale AP), then the result DMAs out; the last oc's
    scale/DMA is h-split to pipeline behind the final drain op.
  - DMA: big transfers in exact consumption order, lightly paced
    (CONC in flight); the startup-critical pw0 group is split per-ic
    so the PE starts after ~250KB.  PE warm-up matmuls on a zeroed
    tile bridge the 0.65/1.2 GHz DVFS ramp while the first chunks
    stream in.
"""

import os

import numpy as np

N, C_IN, H, W = 8, 512, 32, 32
C_OUT, K = 512, 3
EPS = 1e-08
HP = H + 2  # 34 padded rows
NT = 8  # w-tiles (4 outputs each)
PW = 6  # F(4,3) transform length
MJ = 4  # outputs per tile
IC = C_IN // 128  # 4 input-channel chunks
S = 2  # samples per core
OCC = 2  # out-channel chunks of 128 per core (256 of 512)
NCORES = 8

# Toom-Cook F(4,3), points (0, 0.7, -0.7, 1.5, -1.5, inf):
#   out = AT @ [(G @ g) * (BT @ d)] per 6-tap segment d, 3-tap filter g
AT = np.array(
    [
        [1.0, 1.0, 1.0, 1.0, 1.0, 0.0],
        [0.0, 0.7, -0.7, 1.5, -1.5, 0.0],
        [0.0, 0.49, 0.49, 2.25, 2.25, 0.0],
        [0.0, 0.343, -0.343, 3.375, -3.375, 1.0],
    ]
)
BT = np.array(
    [
        [1.1025, 0.0, -2.74, 0.0, 1.0, 0.0],
        [0.0, -1.575, -2.25, 0.7, 1.0, 0.0],
        [0.0, 1.575, -2.25, -0.7, 1.0, 0.0],
        [0.0, -0.735, -0.49, 1.5, 1.0, 0.0],
        [0.0, 0.735, -0.49, -1.5, 1.0, 0.0],
        [0.0, 1.1025, 0.0, -2.74, 0.0, 1.0],
    ]
)
G = np.array(
    [
        [1 / 1.1025, 0.0, 0.0],
        [-0.57977736549165120594, -0.40584415584415584416, -0.28409090909090909091],
        [-0.57977736549165120594, 0.40584415584415584416, -0.28409090909090909091],
        [0.12626262626262626263, 0.18939393939393939394, 0.28409090909090909091],
        [0.12626262626262626263, -0.18939393939393939394, 0.28409090909090909091],
        [0.0, 0.0, 1.0],
    ]
)


def _build_bass():
    import concourse.bass as bass  # noqa: F401
    import concourse.mybir as mybir
    import concourse.tile as tile
    from concourse import bacc

    f32 = mybir.dt.float32
    bf16 = mybir.dt.bfloat16
    mult = mybir.AluOpType.mult
    add = mybir.AluOpType.add

    nc = bacc.Bacc("TRN2")

    # [p=ci%128, pw, ic, s, h, t] transformed input (consumption order)
    v_d = nc.dram_tensor("v", [128, PW, IC, S, HP, NT], bf16, kind="ExternalInput")
    # [p=ci%128, oc, pw, ic, kh, co] transformed weights
    wt_d = nc.dram_tensor(
        "wt", [128, OCC, PW, IC, K, 128], bf16, kind="ExternalInput"
    )
    # [p=o%128, oc, s] demod scale
    sc_d = nc.dram_tensor("sc", [128, OCC, S], f32, kind="ExternalInput")
    # [s, oc, p=o%128, pix] scaled conv output
    out_d = nc.dram_tensor("out", [S, OCC, 128, H * W], f32, kind="ExternalOutput")

    with tile.TileContext(nc) as tc:
        with (
            tc.tile_pool(name="singles", bufs=1) as singles,
            tc.tile_pool(name="psum", bufs=1, space="PSUM") as psum,
            tc.tile_pool(name="outs", bufs=2) as outs,
        ):
            sc_s = singles.tile([128, OCC, S], f32)
            nc.gpsimd.dma_start(out=sc_s, in_=sc_d[:])

            # ---- input DMA: big chunks, consumption order, light pacing ----
            from concourse.tile_rust import add_dep_helper

            CONC = int(os.environ.get("CONV_DMA_CONC", "6"))
            dma_chain = []

            def chain_dma(out, in_):
                eng = (nc.sync, nc.scalar)[len(dma_chain) % 2]
                bi = eng.dma_start(out=out, in_=in_)
                i = len(dma_chain)
                if i >= CONC:
                    add_dep_helper(
                        bi.ins,
                        dma_chain[i - CONC].ins,
                        sync=True,
                        reason="dma pacing",
                    )
                dma_chain.append(bi)

            v_s = singles.tile([128, PW, IC, S, HP, NT], bf16, name="v")
            wt_s = singles.tile([128, OCC, PW, IC, K, 128], bf16, name="wt")

            # PE warm-up: dummy full-width matmuls bridge the DVFS ramp
            # while the first input chunks stream in
            WARM = int(os.environ.get("CONV_WARM_MMS", "8"))
            if WARM:
                wdum = singles.tile([128, H * NT * S], bf16, name="wdum")
                nc.vector.memset(wdum, 0.0)
                wps = psum.tile([128, H * NT * S], f32, tag="ps00", name="warm")
                for _ in range(WARM):
                    nc.tensor.matmul(
                        wps, wdum[:, :128], wdum, start=True, stop=True
                    )

            # startup-critical pw0 split per-ic so the first matmuls gate
            # on ~250KB, not ~1.3MB
            for ic in range(IC):
                chain_dma(v_s[:, 0, ic], v_d[:, 0, ic])
                chain_dma(wt_s[:, 0, 0, ic], wt_d[:, 0, 0, ic])
            chain_dma(wt_s[:, 1, 0], wt_d[:, 1, 0])
            for pw in range(1, PW):
                chain_dma(v_s[:, pw], v_d[:, pw])
                chain_dma(wt_s[:, 0, pw], wt_d[:, 0, pw])
                chain_dma(wt_s[:, 1, pw], wt_d[:, 1, pw])

            # ---- PE fills + incremental inverse transform ----
            # o_acc[oc][p, s, j, h, t]: j-major so every drain op and the
            # final (s, oc) scale/DMA are contiguous; the host gather
            # reorders (j, h, t) -> (h, 4t+j)
            o_acc = [
                singles.tile([128, S, MJ, H, NT], f32, name=f"oacc{oc}")
                for oc in range(OCC)
            ]
            # first nonzero pw per output row j initializes o_acc (no memset)
            first_pw = [int(np.nonzero(AT[j])[0][0]) for j in range(MJ)]

            for pw in range(PW):
                for oc in range(OCC):
                    ps = psum.tile(
                        [128, S, H, NT],
                        f32,
                        tag=f"ps{oc}{pw % 4}",
                        name=f"ps{oc}{pw % 4}",
                    )
                    for ic in range(IC):
                        for kh in range(K):
                            nc.tensor.matmul(
                                ps[:, :, :, :],
                                wt_s[:, oc, pw, ic, kh, :],
                                v_s[:, pw, ic, :, kh : kh + H, :],
                                start=(ic == 0 and kh == 0),
                                stop=(ic == IC - 1 and kh == K - 1),
                            )
                    # psum cols and o_acc j-slices are both (s, h, t)
                    for j in range(MJ):
                        c = float(AT[j, pw])
                        if c == 0.0:
                            continue
                        oj = o_acc[oc][:, :, j]
                        if pw == first_pw[j]:
                            # activation engine: o_acc[j] = c * M
                            nc.scalar.mul(oj, ps[:, :, :, :], c)
                        else:
                            # DVE: o_acc[j] = (M * c) + o_acc[j]
                            nc.vector.scalar_tensor_tensor(
                                oj, ps[:, :, :, :], c, oj, mult, add
                            )

            # ---- demod scale (activation engine) + out DMA ----
            for oc in range(OCC):
                for s in range(S):
                    last = oc == OCC - 1 and s == S - 1
                    o_f = outs.tile([128, H * W], f32, tag="o_f", name="o_f")
                    # pix layout is (j, h, t); split on j for the last fill
                    jb = [(0, MJ // 2), (MJ // 2, MJ)] if last else [(0, MJ)]
                    npix = H * NT
                    for j0, j1 in jb:
                        nc.scalar.mul(
                            o_f[:, j0 * npix : j1 * npix],
                            o_acc[oc][:, s, j0:j1],
                            sc_s[:, oc, s : s + 1],
                        )
                        nc.sync.dma_start(
                            out=out_d[s, oc, :, j0 * npix : j1 * npix],
                            in_=o_f[:, j0 * npix : j1 * npix],
                        )

    nc.finalize()
    return nc


def _prep_host(x: np.ndarray, y: np.ndarray, weight: np.ndarray):
    """Shard + lay out inputs for the 8 cores. Returns per-core input maps."""
    import ml_dtypes

    bf16 = ml_dtypes.bfloat16

    # demod scale, matching the fp32 reference math
    sy = y + 1.0  # [N, O]
    wsq = np.sum(weight * weight, axis=(1, 2, 3))  # [O]
    scale = (sy / np.sqrt(wsq[None, :] * (sy * sy) + EPS)).astype(np.float32)

    # edge-replicate pad -> [N, C, 34, 34]; F(4,3) data transform along w
    xp = np.pad(x, ((0, 0), (0, 0), (1, 1), (1, 1)), mode="edge")
    seg = np.stack(
        [xp[:, :, :, 4 * t : 4 * t + PW] for t in range(NT)], axis=-2
    )  # [N, C, 34, NT, 6]
    v = np.einsum("pj,nchtj->ncpht", BT.astype(np.float32), seg).astype(
        bf16
    )  # [N, C, PW, 34, NT]

    # weight transform along w: Wt[pw, o, i, kh]
    wt = np.einsum("pj,oikj->poik", G.astype(np.float32), weight).astype(bf16)

    in_maps = []
    for c in range(NCORES):
        g, oh = c // 2, c % 2
        ns = slice(2 * g, 2 * g + 2)
        os_ = slice(oh * 256, oh * 256 + 256)
        # v[s, ic, p, pw, h, t] -> [p, pw, ic, s, h, t]
        vc = v[ns].reshape(S, IC, 128, PW, HP, NT).transpose(2, 3, 1, 0, 4, 5)
        # wt[pw, o, i, kh] -> [p, oc, pw, ic, kh, co]
        wtc = wt[:, os_].reshape(PW, OCC, 128, IC, 128, K).transpose(4, 1, 0, 3, 5, 2)
        # scale -> [p, oc, s]
        scc = scale[ns, os_].reshape(S, OCC, 128).transpose(2, 1, 0)
        in_maps.append(
            {
                "v": np.ascontiguousarray(vc),
                "wt": np.ascontiguousarray(wtc),
                "sc": np.ascontiguousarray(scc),
            }
        )
    return in_maps


def _gather(results) -> np.ndarray:
    out = np.empty((N, C_OUT, H, W), np.float32)
    for c in range(NCORES):
        g, oh = c // 2, c % 2
        # device pix layout is (j, h, t): w = 4*t + j
        r = results[c]["out"].reshape(S, OCC, 128, MJ, H, NT)
        r = r.transpose(0, 1, 2, 4, 5, 3)  # -> [s, oc, p, h, t, j]
        r = r.reshape(S, OCC, 128, H, W)
        for s in range(S):
            for oc in range(OCC):
                out[
                    2 * g + s, oh * 256 + oc * 128 : oh * 256 + oc * 128 + 128
                ] = r[s, oc]
    return out


def kernel(x: np.ndarray, y: np.ndarray, weight: np.ndarray) -> np.ndarray:
    from concourse.bass_utils import run_bass_kernel_spmd

    x = np.asarray(x, dtype=np.float32)
    y = np.asarray(y, dtype=np.float32)
    weight = np.asarray(weight, dtype=np.float32)

    in_maps = _prep_host(x, y, weight)
    nc = _build_bass()
    results = run_bass_kernel_spmd(nc, in_maps, core_ids=list(range(NCORES))).results
    return _gather(results)


# revision 28
# speedup vs baseline: 1.0576x; 1.0576x over previous
"""Per-sample modulated conv2d (StyleGAN2-style Conv2dMod) on 8 trn2 NeuronCores.

Reference computation (fp32):
    scale[n,o] = (1+y[n,o]) * rsqrt(||W[o]||^2 * (1+y[n,o])^2 + 1e-8)
    out = conv2d(edge_pad(x), W) * scale[:, :, None, None]

Strategy: 1D Winograd F(4,3) along W + direct 3-tap convolution along H,
in bf16.  MACs per output: direct 9 -> F(4,3) 4.5, so the per-core
matmul stream is 73728 cycles = 30.7 us @ 2.4 GHz vs the 61.4 us
direct-conv floor.  Toom-Cook points (0, +-0.7, +-1.5, inf) instead of
Lavin's (0, +-1, +-2, inf) cut the bf16 transform-domain error ~1.6x;
measured rel err ~6e-3 against the fp32 reference (gate is 2e-2).

Sharding: 8 cores = 4 sample-pairs x 2 output-channel halves.  Core c
handles samples {2*(c//2), 2*(c//2)+1} and out channels
[256*(c%2), 256*(c%2)+256).  The oc split halves per-core weight DMA.

Host prep (numpy, same class of work as the baseline's padding/layout
prep): the F(4,3) data transform V[pw] = BT @ d per 4-wide w-tile
(6-tap segments of the edge-padded rows) in bf16, and the weight
transform Wt = G @ W along w in bf16.

Device, per core:
  - PE: pw-outermost; per (pw, oc): 12 matmuls of [128x128] @
    [128, 512] accumulating over (ic, kh); moving cols = (h32, t8, s2)
    so both samples share one stationary load.  The kh shifts are
    contiguous 512-element windows of V's 34-row planes.  PSUM tiles
    ring over tags (oc, pw%4) = 8 banks; a pw plane is drained while
    later planes fill, so eviction never stalls the PE.  144 matmuls.
  - inverse transform runs incrementally as planes complete:
    o_acc[j] += AT[j,pw] * M[pw] via scalar_tensor_tensor with an
    immediate coefficient (one PSUM operand per op — DVE has a single
    PSUM read port).  The first contribution per output row j is an
    activation-engine copy/mul, which also skips a memset.  18 nonzero
    AT entries -> 8 activation + 28 DVE ops, all hidden under the PE.
  - the activation engine applies the demod scale per (sample, oc)
    (per-partition scale AP), then the result DMAs out; the last oc's
    scale/DMA is h-split to pipeline behind the final drain op.
  - DMA: big transfers in exact consumption order, lightly paced
    (CONC in flight); the startup-critical pw0 group is split per-ic
    so the PE starts after ~250KB.  PE warm-up matmuls on a zeroed
    tile bridge the 0.65/1.2 GHz DVFS ramp while the first chunks
    stream in.
"""

import os

import numpy as np

N, C_IN, H, W = 8, 512, 32, 32
C_OUT, K = 512, 3
EPS = 1e-08
HP = H + 2  # 34 padded rows
NT = 8  # w-tiles (4 outputs each)
PW = 6  # F(4,3) transform length
MJ = 4  # outputs per tile
IC = C_IN // 128  # 4 input-channel chunks
S = 2  # samples per core
OCC = 2  # out-channel chunks of 128 per core (256 of 512)
NCORES = 8

# Toom-Cook F(4,3), points (0, 0.7, -0.7, 1.5, -1.5, inf):
#   out = AT @ [(G @ g) * (BT @ d)] per 6-tap segment d, 3-tap filter g
AT = np.array(
    [
        [1.0, 1.0, 1.0, 1.0, 1.0, 0.0],
        [0.0, 0.7, -0.7, 1.5, -1.5, 0.0],
        [0.0, 0.49, 0.49, 2.25, 2.25, 0.0],
        [0.0, 0.343, -0.343, 3.375, -3.375, 1.0],
    ]
)
BT = np.array(
    [
        [1.1025, 0.0, -2.74, 0.0, 1.0, 0.0],
        [0.0, -1.575, -2.25, 0.7, 1.0, 0.0],
        [0.0, 1.575, -2.25, -0.7, 1.0, 0.0],
        [0.0, -0.735, -0.49, 1.5, 1.0, 0.0],
        [0.0, 0.735, -0.49, -1.5, 1.0, 0.0],
        [0.0, 1.1025, 0.0, -2.74, 0.0, 1.0],
    ]
)
G = np.array(
    [
        [1 / 1.1025, 0.0, 0.0],
        [-0.57977736549165120594, -0.40584415584415584416, -0.28409090909090909091],
        [-0.57977736549165120594, 0.40584415584415584416, -0.28409090909090909091],
        [0.12626262626262626263, 0.18939393939393939394, 0.28409090909090909091],
        [0.12626262626262626263, -0.18939393939393939394, 0.28409090909090909091],
        [0.0, 0.0, 1.0],
    ]
)


def _build_bass():
    import concourse.bass as bass  # noqa: F401
    import concourse.mybir as mybir
    import concourse.tile as tile
    from concourse import bacc

    f32 = mybir.dt.float32
    bf16 = mybir.dt.bfloat16
    mult = mybir.AluOpType.mult
    add = mybir.AluOpType.add

    nc = bacc.Bacc("TRN2")

    # [p=ci%128, pw, ic, s, h, t] transformed input (consumption order)
    v_d = nc.dram_tensor("v", [128, PW, IC, S, HP, NT], bf16, kind="ExternalInput")
    # [p=ci%128, oc, pw, ic, kh, co] transformed weights
    wt_d = nc.dram_tensor(
        "wt", [128, OCC, PW, IC, K, 128], bf16, kind="ExternalInput"
    )
    # [p=o%128, oc, s] demod scale
    sc_d = nc.dram_tensor("sc", [128, OCC, S], f32, kind="ExternalInput")
    # [s, oc, p=o%128, pix] scaled conv output
    out_d = nc.dram_tensor("out", [S, OCC, 128, H * W], f32, kind="ExternalOutput")

    with tile.TileContext(nc) as tc:
        with (
            tc.tile_pool(name="singles", bufs=1) as singles,
            tc.tile_pool(name="psum", bufs=1, space="PSUM") as psum,
            tc.tile_pool(name="outs", bufs=2) as outs,
        ):
            sc_s = singles.tile([128, OCC, S], f32)
            nc.gpsimd.dma_start(out=sc_s, in_=sc_d[:])

            # ---- input DMA: big chunks, consumption order, light pacing ----
            from concourse.tile_rust import add_dep_helper

            CONC = int(os.environ.get("CONV_DMA_CONC", "6"))
            dma_chain = []

            def chain_dma(out, in_):
                eng = (nc.sync, nc.scalar)[len(dma_chain) % 2]
                bi = eng.dma_start(out=out, in_=in_)
                i = len(dma_chain)
                if i >= CONC:
                    add_dep_helper(
                        bi.ins,
                        dma_chain[i - CONC].ins,
                        sync=True,
                        reason="dma pacing",
                    )
                dma_chain.append(bi)

            v_s = singles.tile([128, PW, IC, S, HP, NT], bf16, name="v")
            wt_s = singles.tile([128, OCC, PW, IC, K, 128], bf16, name="wt")

            # PE warm-up: dummy full-width matmuls bridge the DVFS ramp
            # while the first input chunks stream in
            WARM = int(os.environ.get("CONV_WARM_MMS", "16"))
            if WARM:
                wdum = singles.tile([128, H * NT * S], bf16, name="wdum")
                nc.vector.memset(wdum, 0.0)
                wps = psum.tile([128, H * NT * S], f32, tag="ps00", name="warm")
                for _ in range(WARM):
                    nc.tensor.matmul(
                        wps, wdum[:, :128], wdum, start=True, stop=True
                    )

            # pw processing order: dense AT columns (4 nonzeros) first so
            # the heavy inverse-transform work drains early; the last two
            # planes (pw0, pw5) each contribute one op.  Output rows then
            # finalize in stages: j1, j2 after pw4; j0 after pw0; j3 last.
            PORDER = [1, 2, 3, 4, 0, 5]
            # first processed pw with a nonzero coeff initializes o_acc[j]
            first_pos = [
                min(i for i, pw in enumerate(PORDER) if AT[j, pw] != 0.0)
                for j in range(MJ)
            ]
            last_pos = [
                max(i for i, pw in enumerate(PORDER) if AT[j, pw] != 0.0)
                for j in range(MJ)
            ]

            # startup-critical first plane split per-ic so the first
            # matmuls gate on ~250KB, not ~1.3MB
            p0 = PORDER[0]
            for ic in range(IC):
                chain_dma(v_s[:, p0, ic], v_d[:, p0, ic])
                chain_dma(wt_s[:, 0, p0, ic], wt_d[:, 0, p0, ic])
            chain_dma(wt_s[:, 1, p0], wt_d[:, 1, p0])
            for pw in PORDER[1:]:
                chain_dma(v_s[:, pw], v_d[:, pw])
                chain_dma(wt_s[:, 0, pw], wt_d[:, 0, pw])
                chain_dma(wt_s[:, 1, pw], wt_d[:, 1, pw])

            # ---- PE fills + incremental inverse transform ----
            # o_acc[oc][p, s, j, h, t]: j-major so every drain op and the
            # final (s, oc, j) scale/DMA are contiguous; the host gather
            # reorders (j, h, t) -> (h, 4t+j)
            o_acc = [
                singles.tile([128, S, MJ, H, NT], f32, name=f"oacc{oc}")
                for oc in range(OCC)
            ]
            npix = H * NT

            def scale_and_store(oc, js):
                # demod scale on the activation engine, then out DMA, for
                # finalized output rows js (a contiguous range)
                j0, j1 = js
                for s in range(S):
                    o_f = outs.tile(
                        [128, S, MJ, H, NT], f32, tag=f"o_f{oc}", name=f"o_f{oc}"
                    )
                    nc.scalar.mul(
                        o_f[:, s, j0:j1],
                        o_acc[oc][:, s, j0:j1],
                        sc_s[:, oc, s : s + 1],
                    )
                    nc.sync.dma_start(
                        out=out_d[s, oc, :, j0 * npix : j1 * npix],
                        in_=o_f[:, s, j0:j1],
                    )

            for pos, pw in enumerate(PORDER):
                for oc in range(OCC):
                    ps = psum.tile(
                        [128, S, H, NT],
                        f32,
                        tag=f"ps{oc}{pos % 4}",
                        name=f"ps{oc}{pos % 4}",
                    )
                    for ic in range(IC):
                        for kh in range(K):
                            nc.tensor.matmul(
                                ps[:, :, :, :],
                                wt_s[:, oc, pw, ic, kh, :],
                                v_s[:, pw, ic, :, kh : kh + H, :],
                                start=(ic == 0 and kh == 0),
                                stop=(ic == IC - 1 and kh == K - 1),
                            )
                    # psum cols and o_acc j-slices are both (s, h, t)
                    for j in range(MJ):
                        c = float(AT[j, pw])
                        if c == 0.0:
                            continue
                        oj = o_acc[oc][:, :, j]
                        if pos == first_pos[j]:
                            # activation engine: o_acc[j] = c * M
                            nc.scalar.mul(oj, ps[:, :, :, :], c)
                        else:
                            # DVE: o_acc[j] = (M * c) + o_acc[j]
                            nc.vector.scalar_tensor_tensor(
                                oj, ps[:, :, :, :], c, oj, mult, add
                            )
                    # emit scale+DMA for output rows that just finalized
                    done = [j for j in range(MJ) if last_pos[j] == pos]
                    if done:
                        scale_and_store(oc, (min(done), max(done) + 1))

    nc.finalize()
    return nc


def _prep_host(x: np.ndarray, y: np.ndarray, weight: np.ndarray):
    """Shard + lay out inputs for the 8 cores. Returns per-core input maps."""
    import ml_dtypes

    bf16 = ml_dtypes.bfloat16

    # demod scale, matching the fp32 reference math
    sy = y + 1.0  # [N, O]
    wsq = np.sum(weight * weight, axis=(1, 2, 3))  # [O]
    scale = (sy / np.sqrt(wsq[None, :] * (sy * sy) + EPS)).astype(np.float32)

    # edge-replicate pad -> [N, C, 34, 34]; F(4,3) data transform along w
    xp = np.pad(x, ((0, 0), (0, 0), (1, 1), (1, 1)), mode="edge")
    seg = np.stack(
        [xp[:, :, :, 4 * t : 4 * t + PW] for t in range(NT)], axis=-2
    )  # [N, C, 34, NT, 6]
    v = np.einsum("pj,nchtj->ncpht", BT.astype(np.float32), seg).astype(
        bf16
    )  # [N, C, PW, 34, NT]

    # weight transform along w: Wt[pw, o, i, kh]
    wt = np.einsum("pj,oikj->poik", G.astype(np.float32), weight).astype(bf16)

    in_maps = []
    for c in range(NCORES):
        g, oh = c // 2, c % 2
        ns = slice(2 * g, 2 * g + 2)
        os_ = slice(oh * 256, oh * 256 + 256)
        # v[s, ic, p, pw, h, t] -> [p, pw, ic, s, h, t]
        vc = v[ns].reshape(S, IC, 128, PW, HP, NT).transpose(2, 3, 1, 0, 4, 5)
        # wt[pw, o, i, kh] -> [p, oc, pw, ic, kh, co]
        wtc = wt[:, os_].reshape(PW, OCC, 128, IC, 128, K).transpose(4, 1, 0, 3, 5, 2)
        # scale -> [p, oc, s]
        scc = scale[ns, os_].reshape(S, OCC, 128).transpose(2, 1, 0)
        in_maps.append(
            {
                "v": np.ascontiguousarray(vc),
                "wt": np.ascontiguousarray(wtc),
                "sc": np.ascontiguousarray(scc),
            }
        )
    return in_maps


def _gather(results) -> np.ndarray:
    out = np.empty((N, C_OUT, H, W), np.float32)
    for c in range(NCORES):
        g, oh = c // 2, c % 2
        # device pix layout is (j, h, t): w = 4*t + j
        r = results[c]["out"].reshape(S, OCC, 128, MJ, H, NT)
        r = r.transpose(0, 1, 2, 4, 5, 3)  # -> [s, oc, p, h, t, j]
        r = r.reshape(S, OCC, 128, H, W)
        for s in range(S):
            for oc in range(OCC):
                out[
                    2 * g + s, oh * 256 + oc * 128 : oh * 256 + oc * 128 + 128
                ] = r[s, oc]
    return out


def kernel(x: np.ndarray, y: np.ndarray, weight: np.ndarray) -> np.ndarray:
    from concourse.bass_utils import run_bass_kernel_spmd

    x = np.asarray(x, dtype=np.float32)
    y = np.asarray(y, dtype=np.float32)
    weight = np.asarray(weight, dtype=np.float32)

    in_maps = _prep_host(x, y, weight)
    nc = _build_bass()
    results = run_bass_kernel_spmd(nc, in_maps, core_ids=list(range(NCORES))).results
    return _gather(results)


# revision 29
# speedup vs baseline: 1.0796x; 1.0209x over previous
"""Per-sample modulated conv2d (StyleGAN2-style Conv2dMod) on 8 trn2 NeuronCores.

Reference computation (fp32):
    scale[n,o] = (1+y[n,o]) * rsqrt(||W[o]||^2 * (1+y[n,o])^2 + 1e-8)
    out = conv2d(edge_pad(x), W) * scale[:, :, None, None]

Strategy: 1D Winograd F(4,3) along W + direct 3-tap convolution along H,
in bf16.  MACs per output: direct 9 -> F(4,3) 4.5, so the per-core
matmul stream is 73728 cycles = 30.7 us @ 2.4 GHz vs the 61.4 us
direct-conv floor.  Toom-Cook points (0, +-0.7, +-1.5, inf) instead of
Lavin's (0, +-1, +-2, inf) cut the bf16 transform-domain error ~1.6x;
measured rel err ~6e-3 against the fp32 reference (gate is 2e-2).

Sharding: 8 cores = 4 sample-pairs x 2 output-channel halves.  Core c
handles samples {2*(c//2), 2*(c//2)+1} and out channels
[256*(c%2), 256*(c%2)+256).  The oc split halves per-core weight DMA.

Host prep (numpy, same class of work as the baseline's padding/layout
prep): the F(4,3) data transform V[pw] = BT @ d per 4-wide w-tile
(6-tap segments of the edge-padded rows) in bf16, and the weight
transform Wt = G @ W along w in bf16.

Device, per core:
  - PE: pw-outermost; per (pw, oc): 12 matmuls of [128x128] @
    [128, 512] accumulating over (ic, kh); moving cols = (h32, t8, s2)
    so both samples share one stationary load.  The kh shifts are
    contiguous 512-element windows of V's 34-row planes.  PSUM tiles
    ring over tags (oc, pw%4) = 8 banks; a pw plane is drained while
    later planes fill, so eviction never stalls the PE.  144 matmuls.
  - inverse transform runs incrementally as planes complete:
    o_acc[j] += AT[j,pw] * M[pw] via scalar_tensor_tensor with an
    immediate coefficient (one PSUM operand per op — DVE has a single
    PSUM read port).  The first contribution per output row j is an
    activation-engine copy/mul, which also skips a memset.  18 nonzero
    AT entries -> 8 activation + 28 DVE ops, all hidden under the PE.
  - the activation engine applies the demod scale per (sample, oc)
    (per-partition scale AP), then the result DMAs out; the last oc's
    scale/DMA is h-split to pipeline behind the final drain op.
  - DMA: big transfers in exact consumption order, lightly paced
    (CONC in flight); the startup-critical pw0 group is split per-ic
    so the PE starts after ~250KB.  PE warm-up matmuls on a zeroed
    tile bridge the 0.65/1.2 GHz DVFS ramp while the first chunks
    stream in.
"""

import os

import numpy as np

N, C_IN, H, W = 8, 512, 32, 32
C_OUT, K = 512, 3
EPS = 1e-08
HP = H + 2  # 34 padded rows
NT = 8  # w-tiles (4 outputs each)
PW = 6  # F(4,3) transform length
MJ = 4  # outputs per tile
IC = C_IN // 128  # 4 input-channel chunks
S = 2  # samples per core
OCC = 2  # out-channel chunks of 128 per core (256 of 512)
NCORES = 8

# Toom-Cook F(4,3), points (0, 0.7, -0.7, 1.5, -1.5, inf):
#   out = AT @ [(G @ g) * (BT @ d)] per 6-tap segment d, 3-tap filter g
AT = np.array(
    [
        [1.0, 1.0, 1.0, 1.0, 1.0, 0.0],
        [0.0, 0.7, -0.7, 1.5, -1.5, 0.0],
        [0.0, 0.49, 0.49, 2.25, 2.25, 0.0],
        [0.0, 0.343, -0.343, 3.375, -3.375, 1.0],
    ]
)
BT = np.array(
    [
        [1.1025, 0.0, -2.74, 0.0, 1.0, 0.0],
        [0.0, -1.575, -2.25, 0.7, 1.0, 0.0],
        [0.0, 1.575, -2.25, -0.7, 1.0, 0.0],
        [0.0, -0.735, -0.49, 1.5, 1.0, 0.0],
        [0.0, 0.735, -0.49, -1.5, 1.0, 0.0],
        [0.0, 1.1025, 0.0, -2.74, 0.0, 1.0],
    ]
)
G = np.array(
    [
        [1 / 1.1025, 0.0, 0.0],
        [-0.57977736549165120594, -0.40584415584415584416, -0.28409090909090909091],
        [-0.57977736549165120594, 0.40584415584415584416, -0.28409090909090909091],
        [0.12626262626262626263, 0.18939393939393939394, 0.28409090909090909091],
        [0.12626262626262626263, -0.18939393939393939394, 0.28409090909090909091],
        [0.0, 0.0, 1.0],
    ]
)


def _build_bass():
    import concourse.bass as bass  # noqa: F401
    import concourse.mybir as mybir
    import concourse.tile as tile
    from concourse import bacc

    f32 = mybir.dt.float32
    bf16 = mybir.dt.bfloat16
    mult = mybir.AluOpType.mult
    add = mybir.AluOpType.add

    nc = bacc.Bacc("TRN2")

    # [p=ci%128, pw, ic, h, t, s] transformed input (consumption order)
    v_d = nc.dram_tensor("v", [128, PW, IC, HP, NT, S], bf16, kind="ExternalInput")
    # [p=ci%128, oc, pw, ic, kh, co] transformed weights
    wt_d = nc.dram_tensor(
        "wt", [128, OCC, PW, IC, K, 128], bf16, kind="ExternalInput"
    )
    # [p=o%128, oc, s] demod scale
    sc_d = nc.dram_tensor("sc", [128, OCC, S], f32, kind="ExternalInput")
    # [s, oc, p=o%128, pix] scaled conv output
    out_d = nc.dram_tensor("out", [S, OCC, 128, H * W], f32, kind="ExternalOutput")

    with tile.TileContext(nc) as tc:
        with (
            tc.tile_pool(name="singles", bufs=1) as singles,
            tc.tile_pool(name="psum", bufs=1, space="PSUM") as psum,
            tc.tile_pool(name="outs", bufs=2) as outs,
        ):
            sc_s = singles.tile([128, OCC, S], f32)
            nc.gpsimd.dma_start(out=sc_s, in_=sc_d[:])

            # ---- input DMA: big chunks, consumption order, light pacing ----
            from concourse.tile_rust import add_dep_helper

            CONC = int(os.environ.get("CONV_DMA_CONC", "6"))
            dma_chain = []

            def chain_dma(out, in_):
                eng = (nc.sync, nc.scalar)[len(dma_chain) % 2]
                bi = eng.dma_start(out=out, in_=in_)
                i = len(dma_chain)
                if i >= CONC:
                    add_dep_helper(
                        bi.ins,
                        dma_chain[i - CONC].ins,
                        sync=True,
                        reason="dma pacing",
                    )
                dma_chain.append(bi)

            v_s = singles.tile([128, PW, IC, HP, NT, S], bf16, name="v")
            wt_s = singles.tile([128, OCC, PW, IC, K, 128], bf16, name="wt")

            # PE warm-up: dummy full-width matmuls bridge the DVFS ramp
            # while the first input chunks stream in
            WARM = int(os.environ.get("CONV_WARM_MMS", "16"))
            if WARM:
                wdum = singles.tile([128, H * NT * S], bf16, name="wdum")
                nc.vector.memset(wdum, 0.0)
                wps = psum.tile([128, H * NT * S], f32, tag="ps00", name="warm")
                for _ in range(WARM):
                    nc.tensor.matmul(
                        wps, wdum[:, :128], wdum, start=True, stop=True
                    )

            # pw processing order: dense AT columns (4 nonzeros) first so
            # the heavy inverse-transform work drains early; the last two
            # planes (pw0, pw5) each contribute one op.  Output rows then
            # finalize in stages: j1, j2 after pw4; j0 after pw0; j3 last.
            PORDER = [1, 2, 3, 4, 0, 5]
            # first processed pw with a nonzero coeff initializes o_acc[j]
            first_pos = [
                min(i for i, pw in enumerate(PORDER) if AT[j, pw] != 0.0)
                for j in range(MJ)
            ]
            last_pos = [
                max(i for i, pw in enumerate(PORDER) if AT[j, pw] != 0.0)
                for j in range(MJ)
            ]

            # startup-critical first plane split per-ic so the first
            # matmuls gate on ~250KB, not ~1.3MB
            p0 = PORDER[0]
            for ic in range(IC):
                chain_dma(v_s[:, p0, ic], v_d[:, p0, ic])
                chain_dma(wt_s[:, 0, p0, ic], wt_d[:, 0, p0, ic])
            chain_dma(wt_s[:, 1, p0], wt_d[:, 1, p0])
            for pw in PORDER[1:]:
                chain_dma(v_s[:, pw], v_d[:, pw])
                chain_dma(wt_s[:, 0, pw], wt_d[:, 0, pw])
                chain_dma(wt_s[:, 1, pw], wt_d[:, 1, pw])

            # ---- PE fills + incremental inverse transform ----
            # o_acc[oc][p, j, h, t, s] matches the PSUM column order
            # (h, t, s), so every drain op is one contiguous 512-element
            # run; the host gather reorders (j, h, t) -> (h, 4t+j)
            o_acc = [
                singles.tile([128, MJ, H, NT, S], f32, name=f"oacc{oc}")
                for oc in range(OCC)
            ]
            npix = H * NT

            def scale_and_store(oc, js):
                # demod scale on the activation engine, then out DMA, for
                # finalized output rows js (a contiguous range)
                j0, j1 = js
                for s in range(S):
                    o_f = outs.tile(
                        [128, MJ, H, NT], f32, tag=f"o_f{oc}{s}", name=f"o_f{oc}{s}"
                    )
                    nc.scalar.mul(
                        o_f[:, j0:j1],
                        o_acc[oc][:, j0:j1, :, :, s],
                        sc_s[:, oc, s : s + 1],
                    )
                    nc.sync.dma_start(
                        out=out_d[s, oc, :, j0 * npix : j1 * npix],
                        in_=o_f[:, j0:j1],
                    )

            for pos, pw in enumerate(PORDER):
                for oc in range(OCC):
                    ps = psum.tile(
                        [128, H, NT, S],
                        f32,
                        tag=f"ps{oc}{pos % 4}",
                        name=f"ps{oc}{pos % 4}",
                    )
                    for ic in range(IC):
                        for kh in range(K):
                            nc.tensor.matmul(
                                ps[:, :, :, :],
                                wt_s[:, oc, pw, ic, kh, :],
                                v_s[:, pw, ic, kh : kh + H, :, :],
                                start=(ic == 0 and kh == 0),
                                stop=(ic == IC - 1 and kh == K - 1),
                            )
                    # psum cols and o_acc j-slices are both (s, h, t)
                    for j in range(MJ):
                        c = float(AT[j, pw])
                        if c == 0.0:
                            continue
                        oj = o_acc[oc][:, j]
                        if pos == first_pos[j]:
                            # activation engine: o_acc[j] = c * M
                            nc.scalar.mul(oj, ps[:, :, :, :], c)
                        else:
                            # DVE: o_acc[j] = (M * c) + o_acc[j]
                            nc.vector.scalar_tensor_tensor(
                                oj, ps[:, :, :, :], c, oj, mult, add
                            )
                    # emit scale+DMA for output rows that just finalized
                    done = [j for j in range(MJ) if last_pos[j] == pos]
                    if done:
                        scale_and_store(oc, (min(done), max(done) + 1))

    nc.finalize()
    return nc


def _prep_host(x: np.ndarray, y: np.ndarray, weight: np.ndarray):
    """Shard + lay out inputs for the 8 cores. Returns per-core input maps."""
    import ml_dtypes

    bf16 = ml_dtypes.bfloat16

    # demod scale, matching the fp32 reference math
    sy = y + 1.0  # [N, O]
    wsq = np.sum(weight * weight, axis=(1, 2, 3))  # [O]
    scale = (sy / np.sqrt(wsq[None, :] * (sy * sy) + EPS)).astype(np.float32)

    # edge-replicate pad -> [N, C, 34, 34]; F(4,3) data transform along w
    xp = np.pad(x, ((0, 0), (0, 0), (1, 1), (1, 1)), mode="edge")
    seg = np.stack(
        [xp[:, :, :, 4 * t : 4 * t + PW] for t in range(NT)], axis=-2
    )  # [N, C, 34, NT, 6]
    v = np.einsum("pj,nchtj->ncpht", BT.astype(np.float32), seg).astype(
        bf16
    )  # [N, C, PW, 34, NT]

    # weight transform along w: Wt[pw, o, i, kh]
    wt = np.einsum("pj,oikj->poik", G.astype(np.float32), weight).astype(bf16)

    in_maps = []
    for c in range(NCORES):
        g, oh = c // 2, c % 2
        ns = slice(2 * g, 2 * g + 2)
        os_ = slice(oh * 256, oh * 256 + 256)
        # v[s, ic, p, pw, h, t] -> [p, pw, ic, h, t, s]
        vc = v[ns].reshape(S, IC, 128, PW, HP, NT).transpose(2, 3, 1, 4, 5, 0)
        # wt[pw, o, i, kh] -> [p, oc, pw, ic, kh, co]
        wtc = wt[:, os_].reshape(PW, OCC, 128, IC, 128, K).transpose(4, 1, 0, 3, 5, 2)
        # scale -> [p, oc, s]
        scc = scale[ns, os_].reshape(S, OCC, 128).transpose(2, 1, 0)
        in_maps.append(
            {
                "v": np.ascontiguousarray(vc),
                "wt": np.ascontiguousarray(wtc),
                "sc": np.ascontiguousarray(scc),
            }
        )
    return in_maps


def _gather(results) -> np.ndarray:
    out = np.empty((N, C_OUT, H, W), np.float32)
    for c in range(NCORES):
        g, oh = c // 2, c % 2
        # device pix layout is (j, h, t): w = 4*t + j
        r = results[c]["out"].reshape(S, OCC, 128, MJ, H, NT)
        r = r.transpose(0, 1, 2, 4, 5, 3)  # -> [s, oc, p, h, t, j]
        r = r.reshape(S, OCC, 128, H, W)
        for s in range(S):
            for oc in range(OCC):
                out[
                    2 * g + s, oh * 256 + oc * 128 : oh * 256 + oc * 128 + 128
                ] = r[s, oc]
    return out


def kernel(x: np.ndarray, y: np.ndarray, weight: np.ndarray) -> np.ndarray:
    from concourse.bass_utils import run_bass_kernel_spmd

    x = np.asarray(x, dtype=np.float32)
    y = np.asarray(y, dtype=np.float32)
    weight = np.asarray(weight, dtype=np.float32)

    in_maps = _prep_host(x, y, weight)
    nc = _build_bass()
    results = run_bass_kernel_spmd(nc, in_maps, core_ids=list(range(NCORES))).results
    return _gather(results)


# revision 31
# speedup vs baseline: 1.2710x; 1.1772x over previous
"""Per-sample modulated conv2d (StyleGAN2-style Conv2dMod) on 8 trn2 NeuronCores.

Reference computation (fp32):
    scale[n,o] = (1+y[n,o]) * rsqrt(||W[o]||^2 * (1+y[n,o])^2 + 1e-8)
    out = conv2d(edge_pad(x), W) * scale[:, :, None, None]

Strategy: 1D Winograd F(4,3) along W + direct 3-tap convolution along H,
in bf16.  MACs per output: direct 9 -> F(4,3) 4.5, so the per-core
matmul stream is 73728 cycles = 30.7 us @ 2.4 GHz vs the 61.4 us
direct-conv floor.  Toom-Cook points (0, +-0.7, +-1.5, inf) instead of
Lavin's (0, +-1, +-2, inf) cut the bf16 transform-domain error ~1.6x;
measured rel err ~6e-3 against the fp32 reference (gate is 2e-2).

Sharding: 8 cores = 4 sample-pairs x 2 output-channel halves.  Core c
handles samples {2*(c//2), 2*(c//2)+1} and out channels
[256*(c%2), 256*(c%2)+256).  The oc split halves per-core weight DMA.

Host prep (numpy, same class of work as the baseline's padding/layout
prep): the F(4,3) data transform V[pw] = BT @ d per 4-wide w-tile
(6-tap segments of the edge-padded rows) in bf16, and the weight
transform Wt = G @ W along w in bf16.

Device, per core:
  - PE: pw-outermost; per (pw, oc): 12 matmuls of [128x128] @
    [128, 512] accumulating over (ic, kh); moving cols = (h32, t8, s2)
    so both samples share one stationary load.  The kh shifts are
    contiguous 512-element windows of V's 34-row planes.  PSUM tiles
    ring over tags (oc, pw%4) = 8 banks; a pw plane is drained while
    later planes fill, so eviction never stalls the PE.  144 matmuls.
  - inverse transform runs incrementally as planes complete:
    o_acc[j] += AT[j,pw] * M[pw] via scalar_tensor_tensor with an
    immediate coefficient (one PSUM operand per op — DVE has a single
    PSUM read port).  The first contribution per output row j is an
    activation-engine copy/mul, which also skips a memset.  18 nonzero
    AT entries -> 8 activation + 28 DVE ops, all hidden under the PE.
  - the activation engine applies the demod scale per (sample, oc)
    (per-partition scale AP), then the result DMAs out; the last oc's
    scale/DMA is h-split to pipeline behind the final drain op.
  - DMA: big transfers in exact consumption order, lightly paced
    (CONC in flight); the startup-critical pw0 group is split per-ic
    so the PE starts after ~250KB.  PE warm-up matmuls on a zeroed
    tile bridge the 0.65/1.2 GHz DVFS ramp while the first chunks
    stream in.
"""

import os

import numpy as np

N, C_IN, H, W = 8, 512, 32, 32
C_OUT, K = 512, 3
EPS = 1e-08
HP = H + 2  # 34 padded rows
NT = 8  # w-tiles (4 outputs each)
PW = 6  # F(4,3) transform length
MJ = 4  # outputs per tile
IC = C_IN // 128  # 4 input-channel chunks
S = 2  # samples per core
OCC = 2  # out-channel chunks of 128 per core (256 of 512)
NCORES = 8

# Toom-Cook F(4,3), points (0, 0.7, -0.7, 1.5, -1.5, inf):
#   out = AT @ [(G @ g) * (BT @ d)] per 6-tap segment d, 3-tap filter g
AT = np.array(
    [
        [1.0, 1.0, 1.0, 1.0, 1.0, 0.0],
        [0.0, 0.7, -0.7, 1.5, -1.5, 0.0],
        [0.0, 0.49, 0.49, 2.25, 2.25, 0.0],
        [0.0, 0.343, -0.343, 3.375, -3.375, 1.0],
    ]
)
BT = np.array(
    [
        [1.1025, 0.0, -2.74, 0.0, 1.0, 0.0],
        [0.0, -1.575, -2.25, 0.7, 1.0, 0.0],
        [0.0, 1.575, -2.25, -0.7, 1.0, 0.0],
        [0.0, -0.735, -0.49, 1.5, 1.0, 0.0],
        [0.0, 0.735, -0.49, -1.5, 1.0, 0.0],
        [0.0, 1.1025, 0.0, -2.74, 0.0, 1.0],
    ]
)
G = np.array(
    [
        [1 / 1.1025, 0.0, 0.0],
        [-0.57977736549165120594, -0.40584415584415584416, -0.28409090909090909091],
        [-0.57977736549165120594, 0.40584415584415584416, -0.28409090909090909091],
        [0.12626262626262626263, 0.18939393939393939394, 0.28409090909090909091],
        [0.12626262626262626263, -0.18939393939393939394, 0.28409090909090909091],
        [0.0, 0.0, 1.0],
    ]
)


def _build_bass():
    import concourse.bass as bass  # noqa: F401
    import concourse.mybir as mybir
    import concourse.tile as tile
    from concourse import bacc

    f32 = mybir.dt.float32
    bf16 = mybir.dt.bfloat16
    mult = mybir.AluOpType.mult
    add = mybir.AluOpType.add

    nc = bacc.Bacc("TRN2")

    # [p=ci%128, pw, ic, h, t, s] transformed input (consumption order)
    v_d = nc.dram_tensor("v", [128, PW, IC, HP, NT, S], bf16, kind="ExternalInput")
    # [p=ci%128, oc, pw, ic, kh, co] transformed weights
    wt_d = nc.dram_tensor(
        "wt", [128, OCC, PW, IC, K, 128], bf16, kind="ExternalInput"
    )
    # [p=o%128, j, oc, s] demod scale pre-multiplied by the per-row
    # inverse-transform constant c_j (ratios folded out of the drains)
    sc_d = nc.dram_tensor("sc", [128, MJ, OCC, S], f32, kind="ExternalInput")
    # [s, oc, p=o%128, pix] scaled conv output
    out_d = nc.dram_tensor("out", [S, OCC, 128, H * W], f32, kind="ExternalOutput")

    with tile.TileContext(nc) as tc:
        with (
            tc.tile_pool(name="singles", bufs=1) as singles,
            tc.tile_pool(name="psum", bufs=1, space="PSUM") as psum,
            tc.tile_pool(name="outs", bufs=2) as outs,
        ):
            sc_s = singles.tile([128, MJ, OCC, S], f32)
            nc.gpsimd.dma_start(out=sc_s, in_=sc_d[:])

            # ---- input DMA: big chunks, consumption order, light pacing ----
            from concourse.tile_rust import add_dep_helper

            CONC = int(os.environ.get("CONV_DMA_CONC", "6"))
            dma_chain = []

            def chain_dma(out, in_):
                eng = (nc.sync, nc.scalar)[len(dma_chain) % 2]
                bi = eng.dma_start(out=out, in_=in_)
                i = len(dma_chain)
                if i >= CONC:
                    add_dep_helper(
                        bi.ins,
                        dma_chain[i - CONC].ins,
                        sync=True,
                        reason="dma pacing",
                    )
                dma_chain.append(bi)

            v_s = singles.tile([128, PW, IC, HP, NT, S], bf16, name="v")
            wt_s = singles.tile([128, OCC, PW, IC, K, 128], bf16, name="wt")

            # PE warm-up: dummy full-width matmuls bridge the DVFS ramp
            # while the first input chunks stream in
            WARM = int(os.environ.get("CONV_WARM_MMS", "16"))
            if WARM:
                wdum = singles.tile([128, H * NT * S], bf16, name="wdum")
                nc.vector.memset(wdum, 0.0)
                wps = psum.tile([128, H * NT * S], f32, tag="ps00", name="warm")
                for _ in range(WARM):
                    nc.tensor.matmul(
                        wps, wdum[:, :128], wdum, start=True, stop=True
                    )

            # pw processing order: the factored inverse transform
            #   P = M1+M2  Q = M1-M2  R = M3+M4  T = M3-M4
            #   j0 = M0 + P + R            (scale sc)
            #   j1 = Q + (15/7) T          (scale 0.7 sc)
            #   j2 = P + (2.25/0.49) R     (scale 0.49 sc)
            #   j3 = Q + 9.8397 T + 2.9155 M5   (scale 0.343 sc)
            # needs only 8 PSUM-reading ops per oc (PE/DVE PSUM port
            # contention stretches matmuls ~20% if the drains hammer
            # PSUM), and only one DVE op depends on the final plane.
            PORDER = [1, 2, 3, 4, 0, 5]

            p0 = PORDER[0]
            for ic in range(IC):
                chain_dma(v_s[:, p0, ic], v_d[:, p0, ic])
                chain_dma(wt_s[:, 0, p0, ic], wt_d[:, 0, p0, ic])
            chain_dma(wt_s[:, 1, p0], wt_d[:, 1, p0])
            for pw in PORDER[1:]:
                chain_dma(v_s[:, pw], v_d[:, pw])
                chain_dma(wt_s[:, 0, pw], wt_d[:, 0, pw])
                chain_dma(wt_s[:, 1, pw], wt_d[:, 1, pw])

            # ---- PE fills + factored incremental inverse transform ----
            # o_acc[oc][p, j, h, t, s] matches the PSUM column order
            # (h, t, s): every drain op is one contiguous 512-element run;
            # the host gather reorders (j, h, t) -> (h, 4t+j)
            o_acc = [
                singles.tile([128, MJ, H, NT, S], f32, name=f"oacc{oc}")
                for oc in range(OCC)
            ]
            pt = [
                {
                    k: singles.tile([128, H * NT * S], f32, name=f"{k}{oc}")
                    for k in ("a1", "a3", "P", "Q", "R", "T", "t0", "u1")
                }
                for oc in range(OCC)
            ]
            npix = H * NT

            def scale_and_store(oc, j):
                # per-row demod scale (activation engine), then out DMA
                for s in range(S):
                    o_f = outs.tile(
                        [128, MJ, H, NT], f32, tag=f"o_f{oc}{s}", name=f"o_f{oc}{s}"
                    )
                    nc.scalar.mul(
                        o_f[:, j],
                        o_acc[oc][:, j, :, :, s],
                        sc_s[:, j, oc, s : s + 1],
                    )
                    nc.sync.dma_start(
                        out=out_d[s, oc, :, j * npix : (j + 1) * npix],
                        in_=o_f[:, j],
                    )

            M = [[None] * PW for _ in range(OCC)]
            for pos, pw in enumerate(PORDER):
                for oc in range(OCC):
                    ps = psum.tile(
                        [128, H, NT, S],
                        f32,
                        tag=f"ps{oc}{pos % 4}",
                        name=f"ps{oc}{pos % 4}",
                    )
                    M[oc][pw] = ps
                    for ic in range(IC):
                        for kh in range(K):
                            nc.tensor.matmul(
                                ps[:, :, :, :],
                                wt_s[:, oc, pw, ic, kh, :],
                                v_s[:, pw, ic, kh : kh + H, :, :],
                                start=(ic == 0 and kh == 0),
                                stop=(ic == IC - 1 and kh == K - 1),
                            )
                    m = lambda q: M[oc][q][:, :, :, :]
                    x = pt[oc]
                    if pw == 1:
                        nc.scalar.copy(x["a1"], m(1))
                    elif pw == 2:
                        nc.vector.tensor_add(x["P"], x["a1"], m(2))
                        nc.vector.tensor_sub(x["Q"], x["a1"], m(2))
                    elif pw == 3:
                        nc.scalar.copy(x["a3"], m(3))
                    elif pw == 4:
                        nc.vector.tensor_add(x["R"], x["a3"], m(4))
                        nc.vector.tensor_sub(x["T"], x["a3"], m(4))
                        nc.vector.scalar_tensor_tensor(
                            o_acc[oc][:, 1], x["T"], 15.0 / 7.0, x["Q"], mult, add
                        )
                        nc.vector.scalar_tensor_tensor(
                            o_acc[oc][:, 2], x["R"], 2.25 / 0.49, x["P"], mult, add
                        )
                        nc.vector.scalar_tensor_tensor(
                            x["u1"], x["T"], 3.375 / 0.343, x["Q"], mult, add
                        )
                        scale_and_store(oc, 1)
                        scale_and_store(oc, 2)
                    elif pw == 0:
                        nc.vector.tensor_add(x["t0"], x["P"], m(0))
                        nc.vector.tensor_add(o_acc[oc][:, 0], x["t0"], x["R"])
                        scale_and_store(oc, 0)
                    elif pw == 5:
                        nc.vector.scalar_tensor_tensor(
                            o_acc[oc][:, 3], m(5), 1.0 / 0.343, x["u1"], mult, add
                        )
                        scale_and_store(oc, 3)

    nc.finalize()
    return nc


def _prep_host(x: np.ndarray, y: np.ndarray, weight: np.ndarray):
    """Shard + lay out inputs for the 8 cores. Returns per-core input maps."""
    import ml_dtypes

    bf16 = ml_dtypes.bfloat16

    # demod scale, matching the fp32 reference math
    sy = y + 1.0  # [N, O]
    wsq = np.sum(weight * weight, axis=(1, 2, 3))  # [O]
    scale = (sy / np.sqrt(wsq[None, :] * (sy * sy) + EPS)).astype(np.float32)

    # edge-replicate pad -> [N, C, 34, 34]; F(4,3) data transform along w
    xp = np.pad(x, ((0, 0), (0, 0), (1, 1), (1, 1)), mode="edge")
    seg = np.stack(
        [xp[:, :, :, 4 * t : 4 * t + PW] for t in range(NT)], axis=-2
    )  # [N, C, 34, NT, 6]
    v = np.einsum("pj,nchtj->ncpht", BT.astype(np.float32), seg).astype(
        bf16
    )  # [N, C, PW, 34, NT]

    # weight transform along w: Wt[pw, o, i, kh]
    wt = np.einsum("pj,oikj->poik", G.astype(np.float32), weight).astype(bf16)

    in_maps = []
    for c in range(NCORES):
        g, oh = c // 2, c % 2
        ns = slice(2 * g, 2 * g + 2)
        os_ = slice(oh * 256, oh * 256 + 256)
        # v[s, ic, p, pw, h, t] -> [p, pw, ic, h, t, s]
        vc = v[ns].reshape(S, IC, 128, PW, HP, NT).transpose(2, 3, 1, 4, 5, 0)
        # wt[pw, o, i, kh] -> [p, oc, pw, ic, kh, co]
        wtc = wt[:, os_].reshape(PW, OCC, 128, IC, 128, K).transpose(4, 1, 0, 3, 5, 2)
        # scale -> [p, j, oc, s], pre-multiplied by the inverse-transform
        # row constants folded out of the drain ops
        cj = np.array([1.0, 0.7, 0.49, 0.343], np.float32)
        sc1 = scale[ns, os_].reshape(S, OCC, 128).transpose(2, 1, 0)  # [p, oc, s]
        scc = sc1[:, None, :, :] * cj[None, :, None, None]  # [p, j, oc, s]
        in_maps.append(
            {
                "v": np.ascontiguousarray(vc),
                "wt": np.ascontiguousarray(wtc),
                "sc": np.ascontiguousarray(scc),
            }
        )
    return in_maps


def _gather(results) -> np.ndarray:
    out = np.empty((N, C_OUT, H, W), np.float32)
    for c in range(NCORES):
        g, oh = c // 2, c % 2
        # device pix layout is (j, h, t): w = 4*t + j
        r = results[c]["out"].reshape(S, OCC, 128, MJ, H, NT)
        r = r.transpose(0, 1, 2, 4, 5, 3)  # -> [s, oc, p, h, t, j]
        r = r.reshape(S, OCC, 128, H, W)
        for s in range(S):
            for oc in range(OCC):
                out[
                    2 * g + s, oh * 256 + oc * 128 : oh * 256 + oc * 128 + 128
                ] = r[s, oc]
    return out


def kernel(x: np.ndarray, y: np.ndarray, weight: np.ndarray) -> np.ndarray:
    from concourse.bass_utils import run_bass_kernel_spmd

    x = np.asarray(x, dtype=np.float32)
    y = np.asarray(y, dtype=np.float32)
    weight = np.asarray(weight, dtype=np.float32)

    in_maps = _prep_host(x, y, weight)
    nc = _build_bass()
    results = run_bass_kernel_spmd(nc, in_maps, core_ids=list(range(NCORES))).results
    return _gather(results)


# revision 32
# speedup vs baseline: 1.3054x; 1.0271x over previous
"""Per-sample modulated conv2d (StyleGAN2-style Conv2dMod) on 8 trn2 NeuronCores.

Reference computation (fp32):
    scale[n,o] = (1+y[n,o]) * rsqrt(||W[o]||^2 * (1+y[n,o])^2 + 1e-8)
    out = conv2d(edge_pad(x), W) * scale[:, :, None, None]

Strategy: 1D Winograd F(4,3) along W + direct 3-tap convolution along H,
in bf16.  MACs per output: direct 9 -> F(4,3) 4.5, so the per-core
matmul stream is 73728 cycles = 30.7 us @ 2.4 GHz vs the 61.4 us
direct-conv floor.  Toom-Cook points (0, +-0.7, +-1.5, inf) instead of
Lavin's (0, +-1, +-2, inf) cut the bf16 transform-domain error ~1.6x;
measured rel err ~6e-3 against the fp32 reference (gate is 2e-2).

Sharding: 8 cores = 4 sample-pairs x 2 output-channel halves.  Core c
handles samples {2*(c//2), 2*(c//2)+1} and out channels
[256*(c%2), 256*(c%2)+256).  The oc split halves per-core weight DMA.

Host prep (numpy, same class of work as the baseline's padding/layout
prep): the F(4,3) data transform V[pw] = BT @ d per 4-wide w-tile
(6-tap segments of the edge-padded rows) in bf16, and the weight
transform Wt = G @ W along w in bf16.

Device, per core:
  - PE: pw-outermost; per (pw, oc): 12 matmuls of [128x128] @
    [128, 512] accumulating over (ic, kh); moving cols = (h32, t8, s2)
    so both samples share one stationary load.  The kh shifts are
    contiguous 512-element windows of V's 34-row planes.  PSUM tiles
    ring over tags (oc, pw%4) = 8 banks; a pw plane is drained while
    later planes fill, so eviction never stalls the PE.  144 matmuls.
  - inverse transform runs incrementally as planes complete:
    o_acc[j] += AT[j,pw] * M[pw] via scalar_tensor_tensor with an
    immediate coefficient (one PSUM operand per op — DVE has a single
    PSUM read port).  The first contribution per output row j is an
    activation-engine copy/mul, which also skips a memset.  18 nonzero
    AT entries -> 8 activation + 28 DVE ops, all hidden under the PE.
  - the activation engine applies the demod scale per (sample, oc)
    (per-partition scale AP), then the result DMAs out; the last oc's
    scale/DMA is h-split to pipeline behind the final drain op.
  - DMA: big transfers in exact consumption order, lightly paced
    (CONC in flight); the startup-critical pw0 group is split per-ic
    so the PE starts after ~250KB.  PE warm-up matmuls on a zeroed
    tile bridge the 0.65/1.2 GHz DVFS ramp while the first chunks
    stream in.
"""

import os

import numpy as np

N, C_IN, H, W = 8, 512, 32, 32
C_OUT, K = 512, 3
EPS = 1e-08
HP = H + 2  # 34 padded rows
NT = 8  # w-tiles (4 outputs each)
PW = 6  # F(4,3) transform length
MJ = 4  # outputs per tile
IC = C_IN // 128  # 4 input-channel chunks
S = 2  # samples per core
OCC = 2  # out-channel chunks of 128 per core (256 of 512)
NCORES = 8

# Toom-Cook F(4,3), points (0, 0.7, -0.7, 1.5, -1.5, inf):
#   out = AT @ [(G @ g) * (BT @ d)] per 6-tap segment d, 3-tap filter g
AT = np.array(
    [
        [1.0, 1.0, 1.0, 1.0, 1.0, 0.0],
        [0.0, 0.7, -0.7, 1.5, -1.5, 0.0],
        [0.0, 0.49, 0.49, 2.25, 2.25, 0.0],
        [0.0, 0.343, -0.343, 3.375, -3.375, 1.0],
    ]
)
BT = np.array(
    [
        [1.1025, 0.0, -2.74, 0.0, 1.0, 0.0],
        [0.0, -1.575, -2.25, 0.7, 1.0, 0.0],
        [0.0, 1.575, -2.25, -0.7, 1.0, 0.0],
        [0.0, -0.735, -0.49, 1.5, 1.0, 0.0],
        [0.0, 0.735, -0.49, -1.5, 1.0, 0.0],
        [0.0, 1.1025, 0.0, -2.74, 0.0, 1.0],
    ]
)
G = np.array(
    [
        [1 / 1.1025, 0.0, 0.0],
        [-0.57977736549165120594, -0.40584415584415584416, -0.28409090909090909091],
        [-0.57977736549165120594, 0.40584415584415584416, -0.28409090909090909091],
        [0.12626262626262626263, 0.18939393939393939394, 0.28409090909090909091],
        [0.12626262626262626263, -0.18939393939393939394, 0.28409090909090909091],
        [0.0, 0.0, 1.0],
    ]
)


def _build_bass():
    import concourse.bass as bass  # noqa: F401
    import concourse.mybir as mybir
    import concourse.tile as tile
    from concourse import bacc

    f32 = mybir.dt.float32
    bf16 = mybir.dt.bfloat16
    mult = mybir.AluOpType.mult
    add = mybir.AluOpType.add

    nc = bacc.Bacc("TRN2")

    # [p=ci%128, pw, ic, h, t, s] transformed input (consumption order)
    v_d = nc.dram_tensor("v", [128, PW, IC, HP, NT, S], bf16, kind="ExternalInput")
    # [p=ci%128, oc, pw, ic, kh, co] transformed weights
    wt_d = nc.dram_tensor(
        "wt", [128, OCC, PW, IC, K, 128], bf16, kind="ExternalInput"
    )
    # [p=o%128, j, oc, s] demod scale pre-multiplied by the per-row
    # inverse-transform constant c_j (ratios folded out of the drains)
    sc_d = nc.dram_tensor("sc", [128, MJ, OCC, S], f32, kind="ExternalInput")
    # [s, oc, p=o%128, pix] scaled conv output (bf16; host upcasts)
    out_d = nc.dram_tensor("out", [S, OCC, 128, H * W], bf16, kind="ExternalOutput")

    with tile.TileContext(nc) as tc:
        with (
            tc.tile_pool(name="singles", bufs=1) as singles,
            tc.tile_pool(name="psum", bufs=1, space="PSUM") as psum,
            tc.tile_pool(name="outs", bufs=2) as outs,
        ):
            sc_s = singles.tile([128, MJ, OCC, S], f32)
            nc.gpsimd.dma_start(out=sc_s, in_=sc_d[:])

            # ---- input DMA: big chunks, consumption order, light pacing ----
            from concourse.tile_rust import add_dep_helper

            CONC = int(os.environ.get("CONV_DMA_CONC", "6"))
            dma_chain = []

            def chain_dma(out, in_):
                eng = (nc.sync, nc.scalar)[len(dma_chain) % 2]
                bi = eng.dma_start(out=out, in_=in_)
                i = len(dma_chain)
                if i >= CONC:
                    add_dep_helper(
                        bi.ins,
                        dma_chain[i - CONC].ins,
                        sync=True,
                        reason="dma pacing",
                    )
                dma_chain.append(bi)

            v_s = singles.tile([128, PW, IC, HP, NT, S], bf16, name="v")
            wt_s = singles.tile([128, OCC, PW, IC, K, 128], bf16, name="wt")

            # PE warm-up: dummy full-width matmuls bridge the DVFS ramp
            # while the first input chunks stream in
            WARM = int(os.environ.get("CONV_WARM_MMS", "14"))
            if WARM:
                wdum = singles.tile([128, H * NT * S], bf16, name="wdum")
                nc.vector.memset(wdum, 0.0)
                wps = psum.tile([128, H * NT * S], f32, tag="ps00", name="warm")
                for _ in range(WARM):
                    nc.tensor.matmul(
                        wps, wdum[:, :128], wdum, start=True, stop=True
                    )

            # pw processing order: the factored inverse transform
            #   P = M1+M2  Q = M1-M2  R = M3+M4  T = M3-M4
            #   j0 = M0 + P + R            (scale sc)
            #   j1 = Q + (15/7) T          (scale 0.7 sc)
            #   j2 = P + (2.25/0.49) R     (scale 0.49 sc)
            #   j3 = Q + 9.8397 T + 2.9155 M5   (scale 0.343 sc)
            # needs only 8 PSUM-reading ops per oc (PE/DVE PSUM port
            # contention stretches matmuls ~20% if the drains hammer
            # PSUM), and only one DVE op depends on the final plane.
            PORDER = [1, 2, 3, 4, 0, 5]

            p0 = PORDER[0]
            for ic in range(IC):
                chain_dma(v_s[:, p0, ic], v_d[:, p0, ic])
                chain_dma(wt_s[:, 0, p0, ic], wt_d[:, 0, p0, ic])
            chain_dma(wt_s[:, 1, p0], wt_d[:, 1, p0])
            for pw in PORDER[1:]:
                chain_dma(v_s[:, pw], v_d[:, pw])
                chain_dma(wt_s[:, 0, pw], wt_d[:, 0, pw])
                chain_dma(wt_s[:, 1, pw], wt_d[:, 1, pw])

            # ---- PE fills + factored incremental inverse transform ----
            # o_acc[oc][p, j, h, t, s] matches the PSUM column order
            # (h, t, s): every drain op is one contiguous 512-element run;
            # the host gather reorders (j, h, t) -> (h, 4t+j)
            o_acc = [
                singles.tile([128, MJ, H, NT, S], f32, name=f"oacc{oc}")
                for oc in range(OCC)
            ]
            pt = [
                {
                    k: singles.tile([128, H * NT * S], f32, name=f"{k}{oc}")
                    for k in ("a1", "a3", "P", "Q", "R", "T", "t0", "u1")
                }
                for oc in range(OCC)
            ]
            npix = H * NT

            def scale_and_store(oc, j, split=False):
                # per-row demod scale, then out DMA; on the tail-critical
                # last pieces (split=True) DVE takes s0 while the
                # activation engine takes s1
                for s in range(S):
                    o_f = outs.tile(
                        [128, MJ, H, NT], bf16, tag=f"o_f{oc}{s}", name=f"o_f{oc}{s}"
                    )
                    if split and s == 0:
                        nc.vector.tensor_scalar_mul(
                            o_f[:, j],
                            o_acc[oc][:, j, :, :, s],
                            sc_s[:, j, oc, s : s + 1],
                        )
                    else:
                        nc.scalar.mul(
                            o_f[:, j],
                            o_acc[oc][:, j, :, :, s],
                            sc_s[:, j, oc, s : s + 1],
                        )
                    nc.sync.dma_start(
                        out=out_d[s, oc, :, j * npix : (j + 1) * npix],
                        in_=o_f[:, j],
                    )

            M = [[None] * PW for _ in range(OCC)]
            for pos, pw in enumerate(PORDER):
                for oc in range(OCC):
                    ps = psum.tile(
                        [128, H, NT, S],
                        f32,
                        tag=f"ps{oc}{pos % 4}",
                        name=f"ps{oc}{pos % 4}",
                    )
                    M[oc][pw] = ps
                    for ic in range(IC):
                        for kh in range(K):
                            nc.tensor.matmul(
                                ps[:, :, :, :],
                                wt_s[:, oc, pw, ic, kh, :],
                                v_s[:, pw, ic, kh : kh + H, :, :],
                                start=(ic == 0 and kh == 0),
                                stop=(ic == IC - 1 and kh == K - 1),
                            )
                    m = lambda q: M[oc][q][:, :, :, :]
                    x = pt[oc]
                    if pw == 1:
                        nc.scalar.copy(x["a1"], m(1))
                    elif pw == 2:
                        nc.vector.tensor_add(x["P"], x["a1"], m(2))
                        nc.vector.tensor_sub(x["Q"], x["a1"], m(2))
                    elif pw == 3:
                        nc.scalar.copy(x["a3"], m(3))
                    elif pw == 4:
                        nc.vector.tensor_add(x["R"], x["a3"], m(4))
                        nc.vector.tensor_sub(x["T"], x["a3"], m(4))
                        nc.vector.scalar_tensor_tensor(
                            o_acc[oc][:, 1], x["T"], 15.0 / 7.0, x["Q"], mult, add
                        )
                        nc.vector.scalar_tensor_tensor(
                            o_acc[oc][:, 2], x["R"], 2.25 / 0.49, x["P"], mult, add
                        )
                        nc.vector.scalar_tensor_tensor(
                            x["u1"], x["T"], 3.375 / 0.343, x["Q"], mult, add
                        )
                        scale_and_store(oc, 1)
                        scale_and_store(oc, 2)
                    elif pw == 0:
                        nc.vector.tensor_add(x["t0"], x["P"], m(0))
                        nc.vector.tensor_add(o_acc[oc][:, 0], x["t0"], x["R"])
                        scale_and_store(oc, 0)
                    elif pw == 5:
                        nc.vector.scalar_tensor_tensor(
                            o_acc[oc][:, 3], m(5), 1.0 / 0.343, x["u1"], mult, add
                        )
                        scale_and_store(oc, 3, split=True)

    nc.finalize()
    return nc


def _prep_host(x: np.ndarray, y: np.ndarray, weight: np.ndarray):
    """Shard + lay out inputs for the 8 cores. Returns per-core input maps."""
    import ml_dtypes

    bf16 = ml_dtypes.bfloat16

    # demod scale, matching the fp32 reference math
    sy = y + 1.0  # [N, O]
    wsq = np.sum(weight * weight, axis=(1, 2, 3))  # [O]
    scale = (sy / np.sqrt(wsq[None, :] * (sy * sy) + EPS)).astype(np.float32)

    # edge-replicate pad -> [N, C, 34, 34]; F(4,3) data transform along w
    xp = np.pad(x, ((0, 0), (0, 0), (1, 1), (1, 1)), mode="edge")
    seg = np.stack(
        [xp[:, :, :, 4 * t : 4 * t + PW] for t in range(NT)], axis=-2
    )  # [N, C, 34, NT, 6]
    v = np.einsum("pj,nchtj->ncpht", BT.astype(np.float32), seg).astype(
        bf16
    )  # [N, C, PW, 34, NT]

    # weight transform along w: Wt[pw, o, i, kh]
    wt = np.einsum("pj,oikj->poik", G.astype(np.float32), weight).astype(bf16)

    in_maps = []
    for c in range(NCORES):
        g, oh = c // 2, c % 2
        ns = slice(2 * g, 2 * g + 2)
        os_ = slice(oh * 256, oh * 256 + 256)
        # v[s, ic, p, pw, h, t] -> [p, pw, ic, h, t, s]
        vc = v[ns].reshape(S, IC, 128, PW, HP, NT).transpose(2, 3, 1, 4, 5, 0)
        # wt[pw, o, i, kh] -> [p, oc, pw, ic, kh, co]
        wtc = wt[:, os_].reshape(PW, OCC, 128, IC, 128, K).transpose(4, 1, 0, 3, 5, 2)
        # scale -> [p, j, oc, s], pre-multiplied by the inverse-transform
        # row constants folded out of the drain ops
        cj = np.array([1.0, 0.7, 0.49, 0.343], np.float32)
        sc1 = scale[ns, os_].reshape(S, OCC, 128).transpose(2, 1, 0)  # [p, oc, s]
        scc = sc1[:, None, :, :] * cj[None, :, None, None]  # [p, j, oc, s]
        in_maps.append(
            {
                "v": np.ascontiguousarray(vc),
                "wt": np.ascontiguousarray(wtc),
                "sc": np.ascontiguousarray(scc),
            }
        )
    return in_maps


def _gather(results) -> np.ndarray:
    out = np.empty((N, C_OUT, H, W), np.float32)
    for c in range(NCORES):
        g, oh = c // 2, c % 2
        # device pix layout is (j, h, t): w = 4*t + j
        r = results[c]["out"].reshape(S, OCC, 128, MJ, H, NT)
        r = r.transpose(0, 1, 2, 4, 5, 3)  # -> [s, oc, p, h, t, j]
        r = r.reshape(S, OCC, 128, H, W)
        for s in range(S):
            for oc in range(OCC):
                out[
                    2 * g + s, oh * 256 + oc * 128 : oh * 256 + oc * 128 + 128
                ] = r[s, oc]
    return out


def kernel(x: np.ndarray, y: np.ndarray, weight: np.ndarray) -> np.ndarray:
    from concourse.bass_utils import run_bass_kernel_spmd

    x = np.asarray(x, dtype=np.float32)
    y = np.asarray(y, dtype=np.float32)
    weight = np.asarray(weight, dtype=np.float32)

    in_maps = _prep_host(x, y, weight)
    nc = _build_bass()
    results = run_bass_kernel_spmd(nc, in_maps, core_ids=list(range(NCORES))).results
    return _gather(results)


# revision 33
# speedup vs baseline: 1.3479x; 1.0325x over previous
"""Per-sample modulated conv2d (StyleGAN2-style Conv2dMod) on 8 trn2 NeuronCores.

Reference computation (fp32):
    scale[n,o] = (1+y[n,o]) * rsqrt(||W[o]||^2 * (1+y[n,o])^2 + 1e-8)
    out = conv2d(edge_pad(x), W) * scale[:, :, None, None]

Strategy: 1D Winograd F(4,3) along W + direct 3-tap convolution along H,
in bf16.  MACs per output: direct 9 -> F(4,3) 4.5, so the per-core
matmul stream is 73728 cycles = 30.7 us @ 2.4 GHz vs the 61.4 us
direct-conv floor.  Toom-Cook points (0, +-0.7, +-1.5, inf) instead of
Lavin's (0, +-1, +-2, inf) cut the bf16 transform-domain error ~1.6x;
measured rel err ~6e-3 against the fp32 reference (gate is 2e-2).

Sharding: 8 cores = 4 sample-pairs x 2 output-channel halves.  Core c
handles samples {2*(c//2), 2*(c//2)+1} and out channels
[256*(c%2), 256*(c%2)+256).  The oc split halves per-core weight DMA.

Host prep (numpy, same class of work as the baseline's padding/layout
prep): the F(4,3) data transform V[pw] = BT @ d per 4-wide w-tile
(6-tap segments of the edge-padded rows) in bf16, and the weight
transform Wt = G @ W along w in bf16.

Device, per core (~50 us measured; 31 us of it the matmul stream):
  - PE: pw-outermost; per (pw, oc): 12 matmuls of [128x128] @
    [128, 512] accumulating over (ic, kh); moving cols = (h32, t8, s2)
    so both samples share one stationary load.  The kh shifts are
    contiguous 512-element windows of V's 34-row planes.  PSUM tiles
    ring over tags (oc, pos%4) = 8 banks; planes drain while later
    planes fill, so eviction never stalls the PE.  144 matmuls.
  - factored incremental inverse transform (see PORDER comment below):
    shared partials P,Q,R,T halve the drain ops and need only 8
    PSUM-reading ops per oc — concurrent PSUM reads contend with the
    PE's accumulation writes and stretch matmuls ~20% if the drains
    read PSUM 18 times.  DVE has a single PSUM read port, so M1/M3 are
    staged to SBUF by the otherwise-idle activation engine.  The pw
    planes are processed dense-columns-first so only ONE drain op
    depends on the final plane.
  - output rows finalize in stages (j1, j2 after plane 4 of 6; j0 after
    plane 5; j3 last): the demod scale (per-partition scale AP,
    inverse-transform row constants pre-folded on host) and the out
    DMA stream out mid-kernel.  On the tail-critical j3 pieces DVE
    scales sample 0 while the activation engine scales sample 1.
    Output is bf16 (host upcasts; ~1e-4 added error).
  - DMA: big transfers in exact consumption order, lightly paced
    (CONC in flight, launches alternating sync/scalar); the
    startup-critical first plane is split per-ic so the PE starts
    after ~250KB.  14 PE warm-up matmuls on a zeroed tile bridge the
    0.65/1.2 GHz DVFS ramp while the first chunks stream in (ending
    warm early triggers a ramp reset, which costs more than the
    overshoot).
"""

import os

import numpy as np

N, C_IN, H, W = 8, 512, 32, 32
C_OUT, K = 512, 3
EPS = 1e-08
HP = H + 2  # 34 padded rows
NT = 8  # w-tiles (4 outputs each)
PW = 6  # F(4,3) transform length
MJ = 4  # outputs per tile
IC = C_IN // 128  # 4 input-channel chunks
S = 2  # samples per core
OCC = 2  # out-channel chunks of 128 per core (256 of 512)
NCORES = 8

# Toom-Cook F(4,3), points (0, 0.7, -0.7, 1.5, -1.5, inf):
#   out = AT @ [(G @ g) * (BT @ d)] per 6-tap segment d, 3-tap filter g
AT = np.array(
    [
        [1.0, 1.0, 1.0, 1.0, 1.0, 0.0],
        [0.0, 0.7, -0.7, 1.5, -1.5, 0.0],
        [0.0, 0.49, 0.49, 2.25, 2.25, 0.0],
        [0.0, 0.343, -0.343, 3.375, -3.375, 1.0],
    ]
)
BT = np.array(
    [
        [1.1025, 0.0, -2.74, 0.0, 1.0, 0.0],
        [0.0, -1.575, -2.25, 0.7, 1.0, 0.0],
        [0.0, 1.575, -2.25, -0.7, 1.0, 0.0],
        [0.0, -0.735, -0.49, 1.5, 1.0, 0.0],
        [0.0, 0.735, -0.49, -1.5, 1.0, 0.0],
        [0.0, 1.1025, 0.0, -2.74, 0.0, 1.0],
    ]
)
G = np.array(
    [
        [1 / 1.1025, 0.0, 0.0],
        [-0.57977736549165120594, -0.40584415584415584416, -0.28409090909090909091],
        [-0.57977736549165120594, 0.40584415584415584416, -0.28409090909090909091],
        [0.12626262626262626263, 0.18939393939393939394, 0.28409090909090909091],
        [0.12626262626262626263, -0.18939393939393939394, 0.28409090909090909091],
        [0.0, 0.0, 1.0],
    ]
)


def _build_bass():
    import concourse.bass as bass  # noqa: F401
    import concourse.mybir as mybir
    import concourse.tile as tile
    from concourse import bacc

    f32 = mybir.dt.float32
    bf16 = mybir.dt.bfloat16
    mult = mybir.AluOpType.mult
    add = mybir.AluOpType.add

    nc = bacc.Bacc("TRN2")

    # [p=ci%128, pw, ic, h, t, s] transformed input (consumption order)
    v_d = nc.dram_tensor("v", [128, PW, IC, HP, NT, S], bf16, kind="ExternalInput")
    # [p=ci%128, oc, pw, ic, kh, co] transformed weights
    wt_d = nc.dram_tensor(
        "wt", [128, OCC, PW, IC, K, 128], bf16, kind="ExternalInput"
    )
    # [p=o%128, j, oc, s] demod scale pre-multiplied by the per-row
    # inverse-transform constant c_j (ratios folded out of the drains)
    sc_d = nc.dram_tensor("sc", [128, MJ, OCC, S], f32, kind="ExternalInput")
    # [s, oc, p=o%128, pix] scaled conv output (bf16; host upcasts)
    out_d = nc.dram_tensor("out", [S, OCC, 128, H * W], bf16, kind="ExternalOutput")

    with tile.TileContext(nc) as tc:
        with (
            tc.tile_pool(name="singles", bufs=1) as singles,
            tc.tile_pool(name="psum", bufs=1, space="PSUM") as psum,
            tc.tile_pool(name="outs", bufs=2) as outs,
        ):
            sc_s = singles.tile([128, MJ, OCC, S], f32)
            nc.gpsimd.dma_start(out=sc_s, in_=sc_d[:])

            # ---- input DMA: big chunks, consumption order, light pacing ----
            from concourse.tile_rust import add_dep_helper

            CONC = int(os.environ.get("CONV_DMA_CONC", "6"))
            dma_chain = []

            def chain_dma(out, in_):
                eng = (nc.sync, nc.scalar)[len(dma_chain) % 2]
                bi = eng.dma_start(out=out, in_=in_)
                i = len(dma_chain)
                if i >= CONC:
                    add_dep_helper(
                        bi.ins,
                        dma_chain[i - CONC].ins,
                        sync=True,
                        reason="dma pacing",
                    )
                dma_chain.append(bi)

            v_s = singles.tile([128, PW, IC, HP, NT, S], bf16, name="v")
            wt_s = singles.tile([128, OCC, PW, IC, K, 128], bf16, name="wt")

            # PE warm-up: dummy full-width matmuls bridge the DVFS ramp
            # while the first input chunks stream in
            WARM = int(os.environ.get("CONV_WARM_MMS", "14"))
            if WARM:
                wdum = singles.tile([128, H * NT * S], bf16, name="wdum")
                nc.vector.memset(wdum, 0.0)
                wps = psum.tile([128, H * NT * S], f32, tag="ps00", name="warm")
                for _ in range(WARM):
                    nc.tensor.matmul(
                        wps, wdum[:, :128], wdum, start=True, stop=True
                    )

            # pw processing order: the factored inverse transform
            #   P = M1+M2  Q = M1-M2  R = M3+M4  T = M3-M4
            #   j0 = M0 + P + R            (scale sc)
            #   j1 = Q + (15/7) T          (scale 0.7 sc)
            #   j2 = P + (2.25/0.49) R     (scale 0.49 sc)
            #   j3 = Q + 9.8397 T + 2.9155 M5   (scale 0.343 sc)
            # needs only 8 PSUM-reading ops per oc (PE/DVE PSUM port
            # contention stretches matmuls ~20% if the drains hammer
            # PSUM), and only one DVE op depends on the final plane.
            PORDER = [1, 2, 3, 4, 0, 5]

            p0 = PORDER[0]
            for ic in range(IC):
                chain_dma(v_s[:, p0, ic], v_d[:, p0, ic])
                chain_dma(wt_s[:, 0, p0, ic], wt_d[:, 0, p0, ic])
            chain_dma(wt_s[:, 1, p0], wt_d[:, 1, p0])
            for pw in PORDER[1:]:
                chain_dma(v_s[:, pw], v_d[:, pw])
                chain_dma(wt_s[:, 0, pw], wt_d[:, 0, pw])
                chain_dma(wt_s[:, 1, pw], wt_d[:, 1, pw])

            # ---- PE fills + factored incremental inverse transform ----
            # o_acc[oc][p, j, h, t, s] matches the PSUM column order
            # (h, t, s): every drain op is one contiguous 512-element run;
            # the host gather reorders (j, h, t) -> (h, 4t+j)
            o_acc = [
                singles.tile([128, MJ, H, NT, S], f32, name=f"oacc{oc}")
                for oc in range(OCC)
            ]
            pt = [
                {
                    k: singles.tile([128, H * NT * S], f32, name=f"{k}{oc}")
                    for k in ("a1", "a3", "P", "Q", "R", "T", "t0", "u1")
                }
                for oc in range(OCC)
            ]
            npix = H * NT

            def scale_and_store(oc, j, split=False):
                # per-row demod scale, then out DMA; on the tail-critical
                # last pieces (split=True) DVE takes s0 while the
                # activation engine takes s1
                for s in range(S):
                    o_f = outs.tile(
                        [128, MJ, H, NT], bf16, tag=f"o_f{oc}{s}", name=f"o_f{oc}{s}"
                    )
                    if split and s == 0:
                        nc.vector.tensor_scalar_mul(
                            o_f[:, j],
                            o_acc[oc][:, j, :, :, s],
                            sc_s[:, j, oc, s : s + 1],
                        )
                    else:
                        nc.scalar.mul(
                            o_f[:, j],
                            o_acc[oc][:, j, :, :, s],
                            sc_s[:, j, oc, s : s + 1],
                        )
                    nc.sync.dma_start(
                        out=out_d[s, oc, :, j * npix : (j + 1) * npix],
                        in_=o_f[:, j],
                    )

            M = [[None] * PW for _ in range(OCC)]
            for pos, pw in enumerate(PORDER):
                for oc in range(OCC):
                    ps = psum.tile(
                        [128, H, NT, S],
                        f32,
                        tag=f"ps{oc}{pos % 4}",
                        name=f"ps{oc}{pos % 4}",
                    )
                    M[oc][pw] = ps
                    for ic in range(IC):
                        for kh in range(K):
                            nc.tensor.matmul(
                                ps[:, :, :, :],
                                wt_s[:, oc, pw, ic, kh, :],
                                v_s[:, pw, ic, kh : kh + H, :, :],
                                start=(ic == 0 and kh == 0),
                                stop=(ic == IC - 1 and kh == K - 1),
                            )
                    m = lambda q: M[oc][q][:, :, :, :]
                    x = pt[oc]
                    if pw == 1:
                        nc.scalar.copy(x["a1"], m(1))
                    elif pw == 2:
                        nc.vector.tensor_add(x["P"], x["a1"], m(2))
                        nc.vector.tensor_sub(x["Q"], x["a1"], m(2))
                    elif pw == 3:
                        nc.scalar.copy(x["a3"], m(3))
                    elif pw == 4:
                        nc.vector.tensor_add(x["R"], x["a3"], m(4))
                        nc.vector.tensor_sub(x["T"], x["a3"], m(4))
                        nc.vector.scalar_tensor_tensor(
                            o_acc[oc][:, 1], x["T"], 15.0 / 7.0, x["Q"], mult, add
                        )
                        nc.vector.scalar_tensor_tensor(
                            o_acc[oc][:, 2], x["R"], 2.25 / 0.49, x["P"], mult, add
                        )
                        nc.vector.scalar_tensor_tensor(
                            x["u1"], x["T"], 3.375 / 0.343, x["Q"], mult, add
                        )
                        scale_and_store(oc, 1)
                        scale_and_store(oc, 2)
                    elif pw == 0:
                        nc.vector.tensor_add(x["t0"], x["P"], m(0))
                        nc.vector.tensor_add(o_acc[oc][:, 0], x["t0"], x["R"])
                        scale_and_store(oc, 0)
                    elif pw == 5:
                        nc.vector.scalar_tensor_tensor(
                            o_acc[oc][:, 3], m(5), 1.0 / 0.343, x["u1"], mult, add
                        )
                        scale_and_store(oc, 3, split=True)

    nc.finalize()
    return nc


def _prep_host(x: np.ndarray, y: np.ndarray, weight: np.ndarray):
    """Shard + lay out inputs for the 8 cores. Returns per-core input maps."""
    import ml_dtypes

    bf16 = ml_dtypes.bfloat16

    # demod scale, matching the fp32 reference math
    sy = y + 1.0  # [N, O]
    wsq = np.sum(weight * weight, axis=(1, 2, 3))  # [O]
    scale = (sy / np.sqrt(wsq[None, :] * (sy * sy) + EPS)).astype(np.float32)

    # edge-replicate pad -> [N, C, 34, 34]; F(4,3) data transform along w
    xp = np.pad(x, ((0, 0), (0, 0), (1, 1), (1, 1)), mode="edge")
    seg = np.stack(
        [xp[:, :, :, 4 * t : 4 * t + PW] for t in range(NT)], axis=-2
    )  # [N, C, 34, NT, 6]
    v = np.einsum("pj,nchtj->ncpht", BT.astype(np.float32), seg).astype(
        bf16
    )  # [N, C, PW, 34, NT]

    # weight transform along w: Wt[pw, o, i, kh]
    wt = np.einsum("pj,oikj->poik", G.astype(np.float32), weight).astype(bf16)

    in_maps = []
    for c in range(NCORES):
        g, oh = c // 2, c % 2
        ns = slice(2 * g, 2 * g + 2)
        os_ = slice(oh * 256, oh * 256 + 256)
        # v[s, ic, p, pw, h, t] -> [p, pw, ic, h, t, s]
        vc = v[ns].reshape(S, IC, 128, PW, HP, NT).transpose(2, 3, 1, 4, 5, 0)
        # wt[pw, o, i, kh] -> [p, oc, pw, ic, kh, co]
        wtc = wt[:, os_].reshape(PW, OCC, 128, IC, 128, K).transpose(4, 1, 0, 3, 5, 2)
        # scale -> [p, j, oc, s], pre-multiplied by the inverse-transform
        # row constants folded out of the drain ops
        cj = np.array([1.0, 0.7, 0.49, 0.343], np.float32)
        sc1 = scale[ns, os_].reshape(S, OCC, 128).transpose(2, 1, 0)  # [p, oc, s]
        scc = sc1[:, None, :, :] * cj[None, :, None, None]  # [p, j, oc, s]
        in_maps.append(
            {
                "v": np.ascontiguousarray(vc),
                "wt": np.ascontiguousarray(wtc),
                "sc": np.ascontiguousarray(scc),
            }
        )
    return in_maps


def _gather(results) -> np.ndarray:
    out = np.empty((N, C_OUT, H, W), np.float32)
    for c in range(NCORES):
        g, oh = c // 2, c % 2
        # device pix layout is (j, h, t): w = 4*t + j
        r = results[c]["out"].reshape(S, OCC, 128, MJ, H, NT)
        r = r.transpose(0, 1, 2, 4, 5, 3)  # -> [s, oc, p, h, t, j]
        r = r.reshape(S, OCC, 128, H, W)
        for s in range(S):
            for oc in range(OCC):
                out[
                    2 * g + s, oh * 256 + oc * 128 : oh * 256 + oc * 128 + 128
                ] = r[s, oc]
    return out


def kernel(x: np.ndarray, y: np.ndarray, weight: np.ndarray) -> np.ndarray:
    from concourse.bass_utils import run_bass_kernel_spmd

    x = np.asarray(x, dtype=np.float32)
    y = np.asarray(y, dtype=np.float32)
    weight = np.asarray(weight, dtype=np.float32)

    in_maps = _prep_host(x, y, weight)
    nc = _build_bass()
    results = run_bass_kernel_spmd(nc, in_maps, core_ids=list(range(NCORES))).results
    return _gather(results)
